# revision 10
# baseline (speedup 1.0000x reference)
"""Multi-head self-attention (2x2048x1024, 16 heads, causal) on 8 TRN2 cores.

Sharding: core c -> (batch b = c//4, head-group g = c%4). Each core computes
4 heads of one batch: QKV projections for its 256 hidden columns, causal
flash-style attention with full seq; the per-seq-block attention outputs are
AllGathered (1MB bf16) across the batch's 4 cores so each core runs the final
mix matmul for its own 256 output rows with no reduction afterwards.
Host side transposes x / weight blocks so every device DMA is contiguous,
and reassembles the full (2, 2048, 1024) output from the shards.
"""

import numpy as np

import concourse.bass as bass
import concourse.mybir as mybir
import concourse.tile as tile
from concourse import bacc
from concourse.bass_utils import run_bass_kernel_spmd

SEQ = 2048
HID = 1024
NH = 16
HD = 64
NH_LOC = 4          # heads per core
DCOL = NH_LOC * HD  # 256 local head dims
SCALE = 1.0 / (HD ** 0.5)
SJ = 512            # seq block (matmul moving dim / psum free)
NSJ = SEQ // SJ     # 4
SC = 128            # seq chunk (scores psum partition dim)
NDIAG = SJ // SC    # 4
NIC = HID // 128    # 8 contraction chunks
W3 = 3 * DCOL       # concatenated q/k/v weight columns

F32 = mybir.dt.float32
BF16 = mybir.dt.bfloat16
PROBS_DT = BF16


def sj_slice(j):
    return slice(j * SJ, (j + 1) * SJ)


def build_program():
    nc = bacc.Bacc("TRN2", target_bir_lowering=False, debug=False, num_devices=8)

    xT = nc.declare_dram_parameter("xT", [HID, SEQ], BF16, isOutput=False)
    w3T = nc.declare_dram_parameter("w3T", [HID, W3], BF16, isOutput=False)
    wmT = nc.declare_dram_parameter("wmT", [HID, DCOL], BF16, isOutput=False)
    bqk = nc.declare_dram_parameter("bqk", [128, 4], F32, isOutput=False)
    bvB = nc.declare_dram_parameter("bvB", [1, DCOL], F32, isOutput=False)
    bmc = nc.declare_dram_parameter("bmc", [128, 2], F32, isOutput=False)
    masks = nc.declare_dram_parameter("masks", [128, SC], BF16, isOutput=False)
    out = nc.declare_dram_parameter("out", [DCOL, SEQ], F32, isOutput=True)

    att_loc = [nc.dram_tensor(f"att_loc{j}", [DCOL, SJ], BF16) for j in range(NSJ)]
    att_g = [nc.dram_tensor(f"att_g{j}", [HID, SJ], BF16)
             for j in range(NSJ)]
    rec_dram = nc.dram_tensor("rec_dram", [16, SJ], F32)

    with tile.TileContext(nc) as tc:
        with (
            tc.tile_pool(name="persist", bufs=1) as pp,
            tc.tile_pool(name="probs", bufs=8) as probs_pool,
            tc.tile_pool(name="ag", bufs=16) as ag_pool,
            tc.tile_pool(name="work", bufs=3) as wp,
            tc.tile_pool(name="pmm", bufs=2, space="PSUM") as pmm,
            tc.tile_pool(name="ps", bufs=2, space="PSUM") as ps,
            tc.tile_pool(name="pa", bufs=2, space="PSUM") as pa,
        ):
            # ---- persistent SBUF tiles -------------------------------------
            x_t = [pp.tile([128, SEQ], BF16, tag=f"x{i}", name=f"x{i}")
                   for i in range(NIC)]
            w3_t = [pp.tile([128, W3], BF16, tag=f"w3{i}", name=f"w3{i}")
                    for i in range(NIC)]
            wm_t = [pp.tile([128, DCOL], BF16, tag=f"wm{i}", name=f"wm{i}")
                    for i in range(NIC)]
            bqk_t = pp.tile([128, 4], F32, tag="bqk", name="bqk")
            bm_t = pp.tile([128, 2], F32, tag="bm", name="bm")
            bv_bt = pp.tile([128, DCOL], F32, tag="bvB", name="bvB")
            mask_t = pp.tile([128, SC], BF16, tag="mask", name="mask")
            qT = [pp.tile([128, SEQ], BF16, tag=f"qT{c}", name=f"qT{c}")
                  for c in range(2)]
            kT = [pp.tile([128, SEQ], BF16, tag=f"kT{c}", name=f"kT{c}")
                  for c in range(2)]
            vv = [pp.tile([128, NH_LOC, HD + 1], PROBS_DT, tag=f"v{s}",
                          name=f"v{s}") for s in range(SEQ // SC)]

            # ---- input DMAs (x via ACT queue, rest via SP queue) -----------
            for i in range(NIC):
                nc.scalar.dma_start(out=x_t[i][:],
                                    in_=xT[i * 128:(i + 1) * 128, :])
            for i in range(NIC):
                nc.sync.dma_start(out=w3_t[i][:],
                                  in_=w3T[i * 128:(i + 1) * 128, :])
            nc.sync.dma_start(out=bqk_t[:], in_=bqk[:])
            nc.sync.dma_start(out=bv_bt[:],
                              in_=bvB[0:1, :].to_broadcast((128, DCOL)))
            nc.sync.dma_start(out=mask_t[:], in_=masks[:, :])
            for i in range(NIC):
                nc.sync.dma_start(out=wm_t[i][:],
                                  in_=wmT[i * 128:(i + 1) * 128, :])
            nc.sync.dma_start(out=bm_t[:], in_=bmc[:])

            # ---- phases 1-3 interleaved per seq block ----------------------
            # attention for block j needs only q[j], k/v[<=j]; emitting
            # projections and attention per block keeps ScalarE exp running
            # alongside phase-1 matmuls instead of after them.
            def proj_qk(sj):
                for qk in range(2):          # 0 = q, 1 = k
                    dst = (qT, kT)[qk]
                    for oc in range(2):
                        pq = pmm.tile([128, SJ], F32, tag="pmm", name="pq")
                        for ic in range(NIC):
                            nc.tensor.matmul(
                                out=pq[:],
                                lhsT=w3_t[ic][:, qk * DCOL + oc * 128:
                                              qk * DCOL + (oc + 1) * 128],
                                rhs=x_t[ic][:, sj_slice(sj)],
                                start=(ic == 0), stop=(ic == NIC - 1))
                        nc.vector.tensor_scalar_add(
                            out=dst[oc][:, sj_slice(sj)],
                            in0=pq[:],
                            scalar1=bqk_t[:, qk * 2 + oc:qk * 2 + oc + 1])

            def proj_v(sj):
                for s in range(sj * NDIAG, (sj + 1) * NDIAG):
                    nc.vector.memset(vv[s][:, :, HD:HD + 1], 1.0)
                    pv = pmm.tile([128, DCOL], F32, tag="pmm", name="pv")
                    for ic in range(NIC):
                        nc.tensor.matmul(
                            out=pv[:],
                            lhsT=x_t[ic][:, s * SC:(s + 1) * SC],
                            rhs=w3_t[ic][:, 2 * DCOL:3 * DCOL],
                            start=(ic == 0), stop=(ic == NIC - 1))
                    nc.vector.tensor_add(
                        out=vv[s][:, :, 0:HD],
                        in0=pv[:].rearrange("p (h d) -> p h d", h=NH_LOC),
                        in1=bv_bt[:].rearrange("p (h d) -> p h d", h=NH_LOC))

            def attention(j):
                noff = j * NDIAG
                nt = noff + NDIAG
                for ch in range(2):      # head pair (2ch, 2ch+1)
                    prt = []
                    for t in range(nt):
                        co = 0 if t < noff else (t - noff) * SC
                        w = SJ - co
                        # both heads' scoresT chunk via 64x128 PE row tiling
                        sp2 = ps.tile([128, 2 * SJ], F32, tag="ps", name="sp2")
                        nc.tensor.matmul(
                            out=sp2[:, 0:w],
                            lhsT=kT[ch][0:HD, t * SC:(t + 1) * SC],
                            rhs=qT[ch][0:HD, j * SJ + co:(j + 1) * SJ],
                            start=True, stop=True, tile_position=(0, 0))
                        nc.tensor.matmul(
                            out=sp2[:, SJ:SJ + w],
                            lhsT=kT[ch][HD:128, t * SC:(t + 1) * SC],
                            rhs=qT[ch][HD:128, j * SJ + co:(j + 1) * SJ],
                            start=True, stop=True, tile_position=(64, 0))
                        pr = probs_pool.tile([128, 2, SJ], PROBS_DT, tag="pr",
                                             name="pr")
                        nc.scalar.activation(
                            out=pr[:, :, 0:w],
                            in_=sp2[:].rearrange("p (b s) -> p b s", b=2)[:, :, 0:w],
                            func=mybir.ActivationFunctionType.Exp,
                            scale=SCALE)
                        if t >= noff:  # triangular mask on diagonal SC cols
                            nc.vector.tensor_mul(out=pr[:, 0, 0:SC],
                                                 in0=pr[:, 0, 0:SC], in1=mask_t[:])
                            nc.vector.tensor_mul(out=pr[:, 1, 0:SC],
                                                 in0=pr[:, 1, 0:SC], in1=mask_t[:])
                        prt.append((pr, co, w))
                    for hh in range(2):
                        h = 2 * ch + hh
                        att_ps = pa.tile([128, SJ], F32, tag="pa", name="att_ps")
                        for t, (pr, co, w) in enumerate(prt):
                            nc.tensor.matmul(
                                out=att_ps[0:HD + 1, co:co + w],
                                lhsT=vv[t][:, h, :],
                                rhs=pr[:, hh, 0:w],
                                start=(t == 0), stop=(t == nt - 1))
                        # normalize rows by the ones-column sum
                        den = wp.tile([128, SJ], F32, tag="den", name="den")
                        nc.vector.tensor_copy(den[HD:HD + 1, :],
                                              att_ps[HD:HD + 1, :])
                        ridx = h * NSJ + j
                        nc.sync.dma_start(out=rec_dram[ridx:ridx + 1, :],
                                          in_=den[HD:HD + 1, :])
                        rec = wp.tile([128, SJ], F32, tag="rec", name="rec")
                        nc.sync.dma_start(
                            out=rec[0:HD, :],
                            in_=rec_dram[ridx:ridx + 1, :].to_broadcast((HD, SJ)))
                        nc.vector.reciprocal_approx_fast(out=rec[0:HD, :],
                                                         in_=rec[0:HD, :])
                        atn = wp.tile([128, SJ], BF16, tag="atn", name="atn")
                        nc.vector.tensor_mul(out=atn[0:HD, :],
                                             in0=att_ps[0:HD, :],
                                             in1=rec[0:HD, :])
                        nc.sync.dma_start(
                            out=att_loc[j][h * HD:(h + 1) * HD, :],
                            in_=atn[0:HD, :])
                # gather all 16 heads' attention for this seq block
                nc.gpsimd.collective_compute(
                    "AllGather", mybir.AluOpType.bypass,
                    replica_groups=[[0, 1, 2, 3], [4, 5, 6, 7]],
                    ins=[att_loc[j][:]], outs=[att_g[j][:]])
                agt = ag_pool.tile([128, NIC, SJ], BF16, tag="ag",
                                   name=f"ag{j}", bufs=2)
                nc.sync.dma_start(
                    out=agt[:],
                    in_=att_g[j][:].rearrange("(i p) s -> p i s", p=128))
                for oc in range(2):
                    pout = pmm.tile([128, SJ], F32, tag="pmm", name="pout")
                    for ic in range(NIC):
                        nc.tensor.matmul(
                            out=pout[:],
                            lhsT=wm_t[ic][:, oc * 128:(oc + 1) * 128],
                            rhs=agt[:, ic, :],
                            start=(ic == 0), stop=(ic == NIC - 1))
                    osb = wp.tile([128, SJ], F32, tag="osb", name="osb")
                    nc.vector.tensor_scalar_add(out=osb[:], in0=pout[:],
                                                scalar1=bm_t[:, oc:oc + 1])
                    nc.sync.dma_start(
                        out=out[oc * 128:(oc + 1) * 128, sj_slice(j)],
                        in_=osb[:])

            for j in range(NSJ):
                proj_qk(j)
                proj_v(j)
                attention(j)

    nc.compile()
    return nc


_NC_CACHE = None


def _get_program():
    global _NC_CACHE
    if _NC_CACHE is None:
        _NC_CACHE = build_program()
    return _NC_CACHE


def make_in_maps(x, Wq, bq, Wk, bk, Wv, bv, Wm, bm):
    import ml_dtypes
    bf16 = ml_dtypes.bfloat16
    xT = [np.ascontiguousarray(x[b].T).astype(bf16) for b in range(2)]
    p = np.arange(128)[:, None]
    f = np.arange(SC)[None, :]
    masks = (f >= p).astype(bf16)
    in_maps = []
    for core in range(8):
        b, g = core // 4, core % 4
        gr = slice(g * DCOL, (g + 1) * DCOL)
        w3 = np.concatenate([Wq[gr, :].T, Wk[gr, :].T, Wv[gr, :].T],
                            axis=1).astype(bf16)
        bqk_c = np.stack([bq[gr][0:128], bq[gr][128:256],
                          bk[gr][0:128], bk[gr][128:256]],
                         axis=1).astype(np.float32)
        bmc = np.stack([bm[gr][0:128], bm[gr][128:256]],
                       axis=1).astype(np.float32)
        in_maps.append({
            "xT": np.ascontiguousarray(xT[b]),
            "w3T": np.ascontiguousarray(w3),
            "wmT": np.ascontiguousarray(Wm[gr, :].T).astype(bf16),
            "bqk": np.ascontiguousarray(bqk_c),
            "bvB": np.ascontiguousarray(bv[gr].reshape(1, DCOL)).astype(np.float32),
            "bmc": np.ascontiguousarray(bmc),
            "masks": masks,
        })
    return in_maps


def assemble_output(results):
    out = np.empty((2, SEQ, HID), np.float32)
    for b in range(2):
        outT = np.concatenate(
            [results[b * 4 + g]["out"] for g in range(4)], axis=0)
        out[b] = outT.T
    return out


def kernel(x, Wq, bq, Wk, bk, Wv, bv, Wm, bm, _trace=False):
    nc = _get_program()
    in_maps = make_in_maps(np.asarray(x, np.float32), np.asarray(Wq),
                           np.asarray(bq), np.asarray(Wk), np.asarray(bk),
                           np.asarray(Wv), np.asarray(bv), np.asarray(Wm),
                           np.asarray(bm))
    res = run_bass_kernel_spmd(nc, in_maps, list(range(8)), trace=_trace)
    out = assemble_output(res.results)
    if _trace:
        return out, res
    return out


# revision 11
# speedup vs baseline: 1.8483x; 1.8483x over previous
"""Multi-head self-attention (2x2048x1024, 16 heads, causal) on 8 TRN2 cores.

Sharding: core c -> (batch b = c//4, head-group g = c%4). Each core computes
4 heads of one batch: QKV projections for its 256 hidden columns, causal
flash-style attention with full seq; the per-seq-block attention outputs are
AllGathered (1MB bf16) across the batch's 4 cores so each core runs the final
mix matmul for its own 256 output rows with no reduction afterwards.
Host side transposes x / weight blocks so every device DMA is contiguous,
and reassembles the full (2, 2048, 1024) output from the shards.
"""

import numpy as np

import concourse.bass as bass
import concourse.mybir as mybir
import concourse.tile as tile
from concourse import bacc
from concourse.bass_utils import run_bass_kernel_spmd

SEQ = 2048
HID = 1024
NH = 16
HD = 64
NH_LOC = 4          # heads per core
DCOL = NH_LOC * HD  # 256 local head dims
SCALE = 1.0 / (HD ** 0.5)
SJ = 512            # seq block (matmul moving dim / psum free)
NSJ = SEQ // SJ     # 4
SC = 128            # seq chunk (scores psum partition dim)
NDIAG = SJ // SC    # 4
NIC = HID // 128    # 8 contraction chunks
W3 = 3 * DCOL       # concatenated q/k/v weight columns

F32 = mybir.dt.float32
BF16 = mybir.dt.bfloat16
PROBS_DT = BF16


def sj_slice(j):
    return slice(j * SJ, (j + 1) * SJ)


def build_program():
    nc = bacc.Bacc("TRN2", target_bir_lowering=False, debug=False, num_devices=8)

    xT = nc.declare_dram_parameter("xT", [HID, SEQ], BF16, isOutput=False)
    w3T = nc.declare_dram_parameter("w3T", [HID, W3], BF16, isOutput=False)
    wmT = nc.declare_dram_parameter("wmT", [HID, DCOL], BF16, isOutput=False)
    bqk = nc.declare_dram_parameter("bqk", [128, 4], F32, isOutput=False)
    bvB = nc.declare_dram_parameter("bvB", [1, DCOL], F32, isOutput=False)
    bmc = nc.declare_dram_parameter("bmc", [128, 2], F32, isOutput=False)
    masks = nc.declare_dram_parameter("masks", [128, SC], BF16, isOutput=False)
    out = nc.declare_dram_parameter("out", [DCOL, SEQ], F32, isOutput=True)

    att_loc = [nc.dram_tensor(f"att_loc{j}", [DCOL, SJ], BF16) for j in range(NSJ)]
    att_g = [nc.dram_tensor(f"att_g{j}", [HID, SJ], BF16)
             for j in range(NSJ)]
    rec_dram = nc.dram_tensor("rec_dram", [16, SJ], F32)
    warm_in = nc.dram_tensor("warm_in", [1, 16], BF16)
    warm_out = nc.dram_tensor("warm_out", [4, 16], BF16)

    with tile.TileContext(nc) as tc:
        with (
            tc.tile_pool(name="persist", bufs=1) as pp,
            tc.tile_pool(name="probs", bufs=8) as probs_pool,
            tc.tile_pool(name="ag", bufs=16) as ag_pool,
            tc.tile_pool(name="work", bufs=3) as wp,
            tc.tile_pool(name="pmm", bufs=2, space="PSUM") as pmm,
            tc.tile_pool(name="ps", bufs=2, space="PSUM") as ps,
            tc.tile_pool(name="pa", bufs=2, space="PSUM") as pa,
        ):
            # ---- persistent SBUF tiles -------------------------------------
            x_t = [pp.tile([128, SEQ], BF16, tag=f"x{i}", name=f"x{i}")
                   for i in range(NIC)]
            w3_t = [pp.tile([128, W3], BF16, tag=f"w3{i}", name=f"w3{i}")
                    for i in range(NIC)]
            wm_t = [pp.tile([128, DCOL], BF16, tag=f"wm{i}", name=f"wm{i}")
                    for i in range(NIC)]
            bqk_t = pp.tile([128, 4], F32, tag="bqk", name="bqk")
            bm_t = pp.tile([128, 2], F32, tag="bm", name="bm")
            bv_bt = pp.tile([128, DCOL], F32, tag="bvB", name="bvB")
            mask_t = pp.tile([128, SC], BF16, tag="mask", name="mask")
            qT = [pp.tile([128, SEQ], BF16, tag=f"qT{c}", name=f"qT{c}")
                  for c in range(2)]
            kT = [pp.tile([128, SEQ], BF16, tag=f"kT{c}", name=f"kT{c}")
                  for c in range(2)]
            vv = [pp.tile([128, NH_LOC, HD + 1], PROBS_DT, tag=f"v{s}",
                          name=f"v{s}") for s in range(SEQ // SC)]

            # warm-up collective: completes ncfw comm-init before the first
            # real AllGather's doorbell rings (mid-init triggers stall ~340us)
            nc.gpsimd.collective_compute(
                "AllGather", mybir.AluOpType.bypass,
                replica_groups=[[0, 1, 2, 3], [4, 5, 6, 7]],
                ins=[warm_in[:]], outs=[warm_out[:]])

            # ---- input DMAs (x via ACT queue, rest via SP queue) -----------
            for i in range(NIC):
                nc.scalar.dma_start(out=x_t[i][:],
                                    in_=xT[i * 128:(i + 1) * 128, :])
            for i in range(NIC):
                nc.sync.dma_start(out=w3_t[i][:],
                                  in_=w3T[i * 128:(i + 1) * 128, :])
            nc.sync.dma_start(out=bqk_t[:], in_=bqk[:])
            nc.sync.dma_start(out=bv_bt[:],
                              in_=bvB[0:1, :].to_broadcast((128, DCOL)))
            nc.sync.dma_start(out=mask_t[:], in_=masks[:, :])
            for i in range(NIC):
                nc.sync.dma_start(out=wm_t[i][:],
                                  in_=wmT[i * 128:(i + 1) * 128, :])
            nc.sync.dma_start(out=bm_t[:], in_=bmc[:])

            # ---- phases 1-3 interleaved per seq block ----------------------
            # attention for block j needs only q[j], k/v[<=j]; emitting
            # projections and attention per block keeps ScalarE exp running
            # alongside phase-1 matmuls instead of after them.
            def proj_qk(sj):
                for qk in range(2):          # 0 = q, 1 = k
                    dst = (qT, kT)[qk]
                    for oc in range(2):
                        pq = pmm.tile([128, SJ], F32, tag="pmm", name="pq")
                        for ic in range(NIC):
                            nc.tensor.matmul(
                                out=pq[:],
                                lhsT=w3_t[ic][:, qk * DCOL + oc * 128:
                                              qk * DCOL + (oc + 1) * 128],
                                rhs=x_t[ic][:, sj_slice(sj)],
                                start=(ic == 0), stop=(ic == NIC - 1))
                        nc.vector.tensor_scalar_add(
                            out=dst[oc][:, sj_slice(sj)],
                            in0=pq[:],
                            scalar1=bqk_t[:, qk * 2 + oc:qk * 2 + oc + 1])

            def proj_v(sj):
                for s in range(sj * NDIAG, (sj + 1) * NDIAG):
                    nc.vector.memset(vv[s][:, :, HD:HD + 1], 1.0)
                    pv = pmm.tile([128, DCOL], F32, tag="pmm", name="pv")
                    for ic in range(NIC):
                        nc.tensor.matmul(
                            out=pv[:],
                            lhsT=x_t[ic][:, s * SC:(s + 1) * SC],
                            rhs=w3_t[ic][:, 2 * DCOL:3 * DCOL],
                            start=(ic == 0), stop=(ic == NIC - 1))
                    nc.vector.tensor_add(
                        out=vv[s][:, :, 0:HD],
                        in0=pv[:].rearrange("p (h d) -> p h d", h=NH_LOC),
                        in1=bv_bt[:].rearrange("p (h d) -> p h d", h=NH_LOC))

            def attention(j):
                noff = j * NDIAG
                nt = noff + NDIAG
                for ch in range(2):      # head pair (2ch, 2ch+1)
                    prt = []
                    for t in range(nt):
                        co = 0 if t < noff else (t - noff) * SC
                        w = SJ - co
                        # both heads' scoresT chunk via 64x128 PE row tiling
                        sp2 = ps.tile([128, 2 * SJ], F32, tag="ps", name="sp2")
                        nc.tensor.matmul(
                            out=sp2[:, 0:w],
                            lhsT=kT[ch][0:HD, t * SC:(t + 1) * SC],
                            rhs=qT[ch][0:HD, j * SJ + co:(j + 1) * SJ],
                            start=True, stop=True, tile_position=(0, 0))
                        nc.tensor.matmul(
                            out=sp2[:, SJ:SJ + w],
                            lhsT=kT[ch][HD:128, t * SC:(t + 1) * SC],
                            rhs=qT[ch][HD:128, j * SJ + co:(j + 1) * SJ],
                            start=True, stop=True, tile_position=(64, 0))
                        pr = probs_pool.tile([128, 2, SJ], PROBS_DT, tag="pr",
                                             name="pr")
                        nc.scalar.activation(
                            out=pr[:, :, 0:w],
                            in_=sp2[:].rearrange("p (b s) -> p b s", b=2)[:, :, 0:w],
                            func=mybir.ActivationFunctionType.Exp,
                            scale=SCALE)
                        if t >= noff:  # triangular mask on diagonal SC cols
                            nc.vector.tensor_mul(out=pr[:, 0, 0:SC],
                                                 in0=pr[:, 0, 0:SC], in1=mask_t[:])
                            nc.vector.tensor_mul(out=pr[:, 1, 0:SC],
                                                 in0=pr[:, 1, 0:SC], in1=mask_t[:])
                        prt.append((pr, co, w))
                    for hh in range(2):
                        h = 2 * ch + hh
                        att_ps = pa.tile([128, SJ], F32, tag="pa", name="att_ps")
                        for t, (pr, co, w) in enumerate(prt):
                            nc.tensor.matmul(
                                out=att_ps[0:HD + 1, co:co + w],
                                lhsT=vv[t][:, h, :],
                                rhs=pr[:, hh, 0:w],
                                start=(t == 0), stop=(t == nt - 1))
                        # normalize rows by the ones-column sum
                        den = wp.tile([128, SJ], F32, tag="den", name="den")
                        nc.vector.tensor_copy(den[HD:HD + 1, :],
                                              att_ps[HD:HD + 1, :])
                        ridx = h * NSJ + j
                        nc.sync.dma_start(out=rec_dram[ridx:ridx + 1, :],
                                          in_=den[HD:HD + 1, :])
                        rec = wp.tile([128, SJ], F32, tag="rec", name="rec")
                        nc.sync.dma_start(
                            out=rec[0:HD, :],
                            in_=rec_dram[ridx:ridx + 1, :].to_broadcast((HD, SJ)))
                        nc.vector.reciprocal_approx_fast(out=rec[0:HD, :],
                                                         in_=rec[0:HD, :])
                        atn = wp.tile([128, SJ], BF16, tag="atn", name="atn")
                        nc.vector.tensor_mul(out=atn[0:HD, :],
                                             in0=att_ps[0:HD, :],
                                             in1=rec[0:HD, :])
                        nc.sync.dma_start(
                            out=att_loc[j][h * HD:(h + 1) * HD, :],
                            in_=atn[0:HD, :])
                # gather all 16 heads' attention for this seq block
                nc.gpsimd.collective_compute(
                    "AllGather", mybir.AluOpType.bypass,
                    replica_groups=[[0, 1, 2, 3], [4, 5, 6, 7]],
                    ins=[att_loc[j][:]], outs=[att_g[j][:]])
                agt = ag_pool.tile([128, NIC, SJ], BF16, tag="ag",
                                   name=f"ag{j}", bufs=2)
                nc.sync.dma_start(
                    out=agt[:],
                    in_=att_g[j][:].rearrange("(i p) s -> p i s", p=128))
                for oc in range(2):
                    pout = pmm.tile([128, SJ], F32, tag="pmm", name="pout")
                    for ic in range(NIC):
                        nc.tensor.matmul(
                            out=pout[:],
                            lhsT=wm_t[ic][:, oc * 128:(oc + 1) * 128],
                            rhs=agt[:, ic, :],
                            start=(ic == 0), stop=(ic == NIC - 1))
                    osb = wp.tile([128, SJ], F32, tag="osb", name="osb")
                    nc.vector.tensor_scalar_add(out=osb[:], in0=pout[:],
                                                scalar1=bm_t[:, oc:oc + 1])
                    nc.sync.dma_start(
                        out=out[oc * 128:(oc + 1) * 128, sj_slice(j)],
                        in_=osb[:])

            for j in range(NSJ):
                proj_qk(j)
                proj_v(j)
                attention(j)

    nc.compile()
    return nc


_NC_CACHE = None


def _get_program():
    global _NC_CACHE
    if _NC_CACHE is None:
        _NC_CACHE = build_program()
    return _NC_CACHE


def make_in_maps(x, Wq, bq, Wk, bk, Wv, bv, Wm, bm):
    import ml_dtypes
    bf16 = ml_dtypes.bfloat16
    xT = [np.ascontiguousarray(x[b].T).astype(bf16) for b in range(2)]
    p = np.arange(128)[:, None]
    f = np.arange(SC)[None, :]
    masks = (f >= p).astype(bf16)
    in_maps = []
    for core in range(8):
        b, g = core // 4, core % 4
        gr = slice(g * DCOL, (g + 1) * DCOL)
        w3 = np.concatenate([Wq[gr, :].T, Wk[gr, :].T, Wv[gr, :].T],
                            axis=1).astype(bf16)
        bqk_c = np.stack([bq[gr][0:128], bq[gr][128:256],
                          bk[gr][0:128], bk[gr][128:256]],
                         axis=1).astype(np.float32)
        bmc = np.stack([bm[gr][0:128], bm[gr][128:256]],
                       axis=1).astype(np.float32)
        in_maps.append({
            "xT": np.ascontiguousarray(xT[b]),
            "w3T": np.ascontiguousarray(w3),
            "wmT": np.ascontiguousarray(Wm[gr, :].T).astype(bf16),
            "bqk": np.ascontiguousarray(bqk_c),
            "bvB": np.ascontiguousarray(bv[gr].reshape(1, DCOL)).astype(np.float32),
            "bmc": np.ascontiguousarray(bmc),
            "masks": masks,
        })
    return in_maps


def assemble_output(results):
    out = np.empty((2, SEQ, HID), np.float32)
    for b in range(2):
        outT = np.concatenate(
            [results[b * 4 + g]["out"] for g in range(4)], axis=0)
        out[b] = outT.T
    return out


def kernel(x, Wq, bq, Wk, bk, Wv, bv, Wm, bm, _trace=False):
    nc = _get_program()
    in_maps = make_in_maps(np.asarray(x, np.float32), np.asarray(Wq),
                           np.asarray(bq), np.asarray(Wk), np.asarray(bk),
                           np.asarray(Wv), np.asarray(bv), np.asarray(Wm),
                           np.asarray(bm))
    res = run_bass_kernel_spmd(nc, in_maps, list(range(8)), trace=_trace)
    out = assemble_output(res.results)
    if _trace:
        return out, res
    return out


# revision 12
# speedup vs baseline: 2.3799x; 1.2876x over previous
"""Multi-head self-attention (2x2048x1024, 16 heads, causal) on 8 TRN2 cores.

Sharding: core c -> (batch b = c//4, head-group g = c%4). Each core computes
4 heads of one batch: QKV projections for its 256 hidden columns, causal
flash-style attention with full seq; the per-seq-block attention outputs are
AllGathered (1MB bf16) across the batch's 4 cores so each core runs the final
mix matmul for its own 256 output rows with no reduction afterwards.
Host side transposes x / weight blocks so every device DMA is contiguous,
and reassembles the full (2, 2048, 1024) output from the shards.
"""

import numpy as np

import concourse.bass as bass
import concourse.mybir as mybir
import concourse.tile as tile
from concourse import bacc
from concourse.bass_utils import run_bass_kernel_spmd

SEQ = 2048
HID = 1024
NH = 16
HD = 64
NH_LOC = 4          # heads per core
DCOL = NH_LOC * HD  # 256 local head dims
SCALE = 1.0 / (HD ** 0.5)
SJ = 512            # seq block (matmul moving dim / psum free)
NSJ = SEQ // SJ     # 4
SC = 128            # seq chunk (scores psum partition dim)
NDIAG = SJ // SC    # 4
NIC = HID // 128    # 8 contraction chunks
W3 = 3 * DCOL       # concatenated q/k/v weight columns

F32 = mybir.dt.float32
BF16 = mybir.dt.bfloat16
PROBS_DT = BF16


def sj_slice(j):
    return slice(j * SJ, (j + 1) * SJ)


def build_program():
    nc = bacc.Bacc("TRN2", target_bir_lowering=False, debug=False, num_devices=8)

    xT = nc.declare_dram_parameter("xT", [HID, SEQ], BF16, isOutput=False)
    w3T = nc.declare_dram_parameter("w3T", [HID, W3], BF16, isOutput=False)
    wmT = nc.declare_dram_parameter("wmT", [HID, DCOL], BF16, isOutput=False)
    bqk = nc.declare_dram_parameter("bqk", [128, 4], F32, isOutput=False)
    bvB = nc.declare_dram_parameter("bvB", [1, DCOL], F32, isOutput=False)
    bmc = nc.declare_dram_parameter("bmc", [128, 2], F32, isOutput=False)
    masks = nc.declare_dram_parameter("masks", [128, SC], BF16, isOutput=False)
    out = nc.declare_dram_parameter("out", [DCOL, SEQ], F32, isOutput=True)

    att_loc = [nc.dram_tensor(f"att_loc{j}", [DCOL, SJ], BF16) for j in range(NSJ)]
    att_g = [nc.dram_tensor(f"att_g{j}", [HID, SJ], BF16)
             for j in range(NSJ)]
    rec_dram = nc.dram_tensor("rec_dram", [16, SJ], F32)
    warm_in = nc.dram_tensor("warm_in", [1, 16], BF16)
    warm_out = nc.dram_tensor("warm_out", [4, 16], BF16)

    with tile.TileContext(nc) as tc:
        with (
            tc.tile_pool(name="persist", bufs=1) as pp,
            tc.tile_pool(name="probs", bufs=16) as probs_pool,
            tc.tile_pool(name="ag", bufs=16) as ag_pool,
            tc.tile_pool(name="work", bufs=3) as wp,
            tc.tile_pool(name="pmm", bufs=2, space="PSUM") as pmm,
            tc.tile_pool(name="ps", bufs=2, space="PSUM") as ps,
            tc.tile_pool(name="pa", bufs=2, space="PSUM") as pa,
        ):
            # ---- persistent SBUF tiles -------------------------------------
            x_t = [pp.tile([128, SEQ], BF16, tag=f"x{i}", name=f"x{i}")
                   for i in range(NIC)]
            w3_t = [pp.tile([128, W3], BF16, tag=f"w3{i}", name=f"w3{i}")
                    for i in range(NIC)]
            wm_t = [pp.tile([128, DCOL], BF16, tag=f"wm{i}", name=f"wm{i}")
                    for i in range(NIC)]
            bqk_t = pp.tile([128, 4], F32, tag="bqk", name="bqk")
            bm_t = pp.tile([128, 2], F32, tag="bm", name="bm")
            bv_bt = pp.tile([128, DCOL], F32, tag="bvB", name="bvB")
            mask_t = pp.tile([128, SC], BF16, tag="mask", name="mask")
            qT = [pp.tile([128, SEQ], BF16, tag=f"qT{c}", name=f"qT{c}")
                  for c in range(2)]
            kT = [pp.tile([128, SEQ], BF16, tag=f"kT{c}", name=f"kT{c}")
                  for c in range(2)]
            vv = [pp.tile([128, NH_LOC, HD + 1], PROBS_DT, tag=f"v{s}",
                          name=f"v{s}") for s in range(SEQ // SC)]

            # warm-up collective: completes ncfw comm-init before the first
            # real AllGather's doorbell rings (mid-init triggers stall ~340us)
            nc.gpsimd.collective_compute(
                "AllGather", mybir.AluOpType.bypass,
                replica_groups=[[0, 1, 2, 3], [4, 5, 6, 7]],
                ins=[warm_in[:]], outs=[warm_out[:]])

            # ---- input DMAs (x via ACT queue, rest via SP queue) -----------
            for i in range(NIC):
                nc.scalar.dma_start(out=x_t[i][:],
                                    in_=xT[i * 128:(i + 1) * 128, :])
            for i in range(NIC):
                nc.sync.dma_start(out=w3_t[i][:],
                                  in_=w3T[i * 128:(i + 1) * 128, :])
            nc.sync.dma_start(out=bqk_t[:], in_=bqk[:])
            nc.sync.dma_start(out=bv_bt[:],
                              in_=bvB[0:1, :].to_broadcast((128, DCOL)))
            nc.sync.dma_start(out=mask_t[:], in_=masks[:, :])
            for i in range(NIC):
                nc.sync.dma_start(out=wm_t[i][:],
                                  in_=wmT[i * 128:(i + 1) * 128, :])
            nc.sync.dma_start(out=bm_t[:], in_=bmc[:])

            # ---- phases 1-3 interleaved per seq block ----------------------
            # attention for block j needs only q[j], k/v[<=j]; emitting
            # projections and attention per block keeps ScalarE exp running
            # alongside phase-1 matmuls instead of after them.
            def proj_qk(sj):
                for qk in range(2):          # 0 = q, 1 = k
                    dst = (qT, kT)[qk]
                    for oc in range(2):
                        pq = pmm.tile([128, SJ], F32, tag="pmm", name="pq")
                        for ic in range(NIC):
                            nc.tensor.matmul(
                                out=pq[:],
                                lhsT=w3_t[ic][:, qk * DCOL + oc * 128:
                                              qk * DCOL + (oc + 1) * 128],
                                rhs=x_t[ic][:, sj_slice(sj)],
                                start=(ic == 0), stop=(ic == NIC - 1))
                        nc.vector.tensor_scalar_add(
                            out=dst[oc][:, sj_slice(sj)],
                            in0=pq[:],
                            scalar1=bqk_t[:, qk * 2 + oc:qk * 2 + oc + 1])

            def proj_v(sj):
                for s in range(sj * NDIAG, (sj + 1) * NDIAG):
                    nc.vector.memset(vv[s][:, :, HD:HD + 1], 1.0)
                    pv = pmm.tile([128, DCOL], F32, tag="pmm", name="pv")
                    for ic in range(NIC):
                        nc.tensor.matmul(
                            out=pv[:],
                            lhsT=x_t[ic][:, s * SC:(s + 1) * SC],
                            rhs=w3_t[ic][:, 2 * DCOL:3 * DCOL],
                            start=(ic == 0), stop=(ic == NIC - 1))
                    nc.vector.tensor_add(
                        out=vv[s][:, :, 0:HD],
                        in0=pv[:].rearrange("p (h d) -> p h d", h=NH_LOC),
                        in1=bv_bt[:].rearrange("p (h d) -> p h d", h=NH_LOC))

            def attention(j):
                noff = j * NDIAG
                nt = noff + NDIAG
                for ch in range(2):      # head pair (2ch, 2ch+1)
                    prt = []
                    for t in range(nt):
                        co = 0 if t < noff else (t - noff) * SC
                        w = SJ - co
                        # both heads' scoresT chunk via 64x128 PE row tiling
                        sp2 = ps.tile([128, 2 * SJ], F32, tag="ps", name="sp2")
                        nc.tensor.matmul(
                            out=sp2[:, 0:w],
                            lhsT=kT[ch][0:HD, t * SC:(t + 1) * SC],
                            rhs=qT[ch][0:HD, j * SJ + co:(j + 1) * SJ],
                            start=True, stop=True, tile_position=(0, 0))
                        nc.tensor.matmul(
                            out=sp2[:, SJ:SJ + w],
                            lhsT=kT[ch][HD:128, t * SC:(t + 1) * SC],
                            rhs=qT[ch][HD:128, j * SJ + co:(j + 1) * SJ],
                            start=True, stop=True, tile_position=(64, 0))
                        pr = probs_pool.tile([128, 2, SJ], PROBS_DT, tag="pr",
                                             name="pr")
                        nc.scalar.activation(
                            out=pr[:, :, 0:w],
                            in_=sp2[:].rearrange("p (b s) -> p b s", b=2)[:, :, 0:w],
                            func=mybir.ActivationFunctionType.Exp,
                            scale=SCALE)
                        if t >= noff:  # triangular mask on diagonal SC cols
                            nc.vector.tensor_mul(out=pr[:, 0, 0:SC],
                                                 in0=pr[:, 0, 0:SC], in1=mask_t[:])
                            nc.vector.tensor_mul(out=pr[:, 1, 0:SC],
                                                 in0=pr[:, 1, 0:SC], in1=mask_t[:])
                        prt.append((pr, co, w))
                    for hh in range(2):
                        h = 2 * ch + hh
                        att_ps = pa.tile([128, SJ], F32, tag="pa", name="att_ps")
                        for t, (pr, co, w) in enumerate(prt):
                            nc.tensor.matmul(
                                out=att_ps[0:HD + 1, co:co + w],
                                lhsT=vv[t][:, h, :],
                                rhs=pr[:, hh, 0:w],
                                start=(t == 0), stop=(t == nt - 1))
                        # normalize rows by the ones-column sum
                        den = wp.tile([128, SJ], F32, tag="den", name="den")
                        nc.vector.tensor_copy(den[HD:HD + 1, :],
                                              att_ps[HD:HD + 1, :])
                        ridx = h * NSJ + j
                        nc.sync.dma_start(out=rec_dram[ridx:ridx + 1, :],
                                          in_=den[HD:HD + 1, :])
                        rec = wp.tile([128, SJ], F32, tag="rec", name="rec")
                        nc.sync.dma_start(
                            out=rec[0:HD, :],
                            in_=rec_dram[ridx:ridx + 1, :].to_broadcast((HD, SJ)))
                        nc.vector.reciprocal_approx_fast(out=rec[0:HD, :],
                                                         in_=rec[0:HD, :])
                        atn = wp.tile([128, SJ], BF16, tag="atn", name="atn")
                        nc.vector.tensor_mul(out=atn[0:HD, :],
                                             in0=att_ps[0:HD, :],
                                             in1=rec[0:HD, :])
                        nc.sync.dma_start(
                            out=att_loc[j][h * HD:(h + 1) * HD, :],
                            in_=atn[0:HD, :])
                # gather all 16 heads' attention for this seq block
                nc.gpsimd.collective_compute(
                    "AllGather", mybir.AluOpType.bypass,
                    replica_groups=[[0, 1, 2, 3], [4, 5, 6, 7]],
                    ins=[att_loc[j][:]], outs=[att_g[j][:]])
                agt = ag_pool.tile([128, NIC, SJ], BF16, tag="ag",
                                   name=f"ag{j}", bufs=2)
                nc.sync.dma_start(
                    out=agt[:],
                    in_=att_g[j][:].rearrange("(i p) s -> p i s", p=128))
                for oc in range(2):
                    pout = pa.tile([128, SJ], F32, tag="pa", name="pout")
                    for ic in range(NIC):
                        nc.tensor.matmul(
                            out=pout[:],
                            lhsT=wm_t[ic][:, oc * 128:(oc + 1) * 128],
                            rhs=agt[:, ic, :],
                            start=(ic == 0), stop=(ic == NIC - 1))
                    osb = wp.tile([128, SJ], F32, tag="osb", name="osb")
                    nc.vector.tensor_scalar_add(out=osb[:], in0=pout[:],
                                                scalar1=bm_t[:, oc:oc + 1])
                    nc.sync.dma_start(
                        out=out[oc * 128:(oc + 1) * 128, sj_slice(j)],
                        in_=osb[:])

            for j in range(NSJ):
                proj_qk(j)
                proj_v(j)
                attention(j)

    nc.compile()
    return nc


_NC_CACHE = None


def _get_program():
    global _NC_CACHE
    if _NC_CACHE is None:
        _NC_CACHE = build_program()
    return _NC_CACHE


def make_in_maps(x, Wq, bq, Wk, bk, Wv, bv, Wm, bm):
    import ml_dtypes
    bf16 = ml_dtypes.bfloat16
    xT = [np.ascontiguousarray(x[b].T).astype(bf16) for b in range(2)]
    p = np.arange(128)[:, None]
    f = np.arange(SC)[None, :]
    masks = (f >= p).astype(bf16)
    in_maps = []
    for core in range(8):
        b, g = core // 4, core % 4
        gr = slice(g * DCOL, (g + 1) * DCOL)
        w3 = np.concatenate([Wq[gr, :].T, Wk[gr, :].T, Wv[gr, :].T],
                            axis=1).astype(bf16)
        bqk_c = np.stack([bq[gr][0:128], bq[gr][128:256],
                          bk[gr][0:128], bk[gr][128:256]],
                         axis=1).astype(np.float32)
        bmc = np.stack([bm[gr][0:128], bm[gr][128:256]],
                       axis=1).astype(np.float32)
        in_maps.append({
            "xT": np.ascontiguousarray(xT[b]),
            "w3T": np.ascontiguousarray(w3),
            "wmT": np.ascontiguousarray(Wm[gr, :].T).astype(bf16),
            "bqk": np.ascontiguousarray(bqk_c),
            "bvB": np.ascontiguousarray(bv[gr].reshape(1, DCOL)).astype(np.float32),
            "bmc": np.ascontiguousarray(bmc),
            "masks": masks,
        })
    return in_maps


def assemble_output(results):
    out = np.empty((2, SEQ, HID), np.float32)
    for b in range(2):
        outT = np.concatenate(
            [results[b * 4 + g]["out"] for g in range(4)], axis=0)
        out[b] = outT.T
    return out


def kernel(x, Wq, bq, Wk, bk, Wv, bv, Wm, bm, _trace=False):
    nc = _get_program()
    in_maps = make_in_maps(np.asarray(x, np.float32), np.asarray(Wq),
                           np.asarray(bq), np.asarray(Wk), np.asarray(bk),
                           np.asarray(Wv), np.asarray(bv), np.asarray(Wm),
                           np.asarray(bm))
    res = run_bass_kernel_spmd(nc, in_maps, list(range(8)), trace=_trace)
    out = assemble_output(res.results)
    if _trace:
        return out, res
    return out


# revision 13
# speedup vs baseline: 2.6729x; 1.1231x over previous
"""Multi-head self-attention (2x2048x1024, 16 heads, causal) on 8 TRN2 cores.

Sharding: core c -> (batch b = c//4, head-group g = c%4). Each core computes
4 heads of one batch: QKV projections for its 256 hidden columns, causal
flash-style attention with full seq; the per-seq-block attention outputs are
AllGathered (1MB bf16) across the batch's 4 cores so each core runs the final
mix matmul for its own 256 output rows with no reduction afterwards.
Host side transposes x / weight blocks so every device DMA is contiguous,
and reassembles the full (2, 2048, 1024) output from the shards.
"""

import numpy as np

import concourse.bass as bass
import concourse.mybir as mybir
import concourse.tile as tile
from concourse import bacc
from concourse.bass_utils import run_bass_kernel_spmd

SEQ = 2048
HID = 1024
NH = 16
HD = 64
NH_LOC = 4          # heads per core
DCOL = NH_LOC * HD  # 256 local head dims
SCALE = 1.0 / (HD ** 0.5)
SJ = 512            # seq block (matmul moving dim / psum free)
NSJ = SEQ // SJ     # 4
SC = 128            # seq chunk (scores psum partition dim)
NDIAG = SJ // SC    # 4
NIC = HID // 128    # 8 contraction chunks
W3 = 3 * DCOL       # concatenated q/k/v weight columns

F32 = mybir.dt.float32
BF16 = mybir.dt.bfloat16
PROBS_DT = BF16


def sj_slice(j):
    return slice(j * SJ, (j + 1) * SJ)


def build_program():
    nc = bacc.Bacc("TRN2", target_bir_lowering=False, debug=False, num_devices=8)

    xT = nc.declare_dram_parameter("xT", [HID, SEQ], BF16, isOutput=False)
    w3T = nc.declare_dram_parameter("w3T", [HID, W3], BF16, isOutput=False)
    wmT = nc.declare_dram_parameter("wmT", [HID, DCOL], BF16, isOutput=False)
    bqk = nc.declare_dram_parameter("bqk", [128, 4], F32, isOutput=False)
    bvB = nc.declare_dram_parameter("bvB", [1, DCOL], F32, isOutput=False)
    bmc = nc.declare_dram_parameter("bmc", [128, 2], F32, isOutput=False)
    masks = nc.declare_dram_parameter("masks", [128, SC], BF16, isOutput=False)
    out = nc.declare_dram_parameter("out", [DCOL, SEQ], F32, isOutput=True)

    att_loc = [nc.dram_tensor(f"att_loc{j}", [DCOL, SJ], BF16) for j in range(NSJ)]
    att_g = [nc.dram_tensor(f"att_g{j}", [HID, SJ], BF16)
             for j in range(NSJ)]
    rec_dram = nc.dram_tensor("rec_dram", [16, SJ], F32)
    warm_in = nc.dram_tensor("warm_in", [1, 16], BF16)
    warm_out = nc.dram_tensor("warm_out", [4, 16], BF16)

    with tile.TileContext(nc) as tc:
        with (
            tc.tile_pool(name="persist", bufs=1) as pp,
            tc.tile_pool(name="probs", bufs=16) as probs_pool,
            tc.tile_pool(name="ag", bufs=16) as ag_pool,
            tc.tile_pool(name="work", bufs=3) as wp,
            tc.tile_pool(name="pmm", bufs=2, space="PSUM") as pmm,
            tc.tile_pool(name="ps", bufs=2, space="PSUM") as ps,
            tc.tile_pool(name="pa", bufs=2, space="PSUM") as pa,
        ):
            # ---- persistent SBUF tiles -------------------------------------
            x_t = [pp.tile([128, SEQ], BF16, tag=f"x{i}", name=f"x{i}")
                   for i in range(NIC)]
            w3_t = [pp.tile([128, W3], BF16, tag=f"w3{i}", name=f"w3{i}")
                    for i in range(NIC)]
            wm_t = [pp.tile([128, DCOL], BF16, tag=f"wm{i}", name=f"wm{i}")
                    for i in range(NIC)]
            bqk_t = pp.tile([128, 4], F32, tag="bqk", name="bqk")
            bm_t = pp.tile([128, 2], F32, tag="bm", name="bm")
            bv_bt = pp.tile([128, DCOL], F32, tag="bvB", name="bvB")
            mask_t = pp.tile([128, SC], BF16, tag="mask", name="mask")
            qT = [pp.tile([128, SEQ], BF16, tag=f"qT{c}", name=f"qT{c}")
                  for c in range(2)]
            kT = [pp.tile([128, SEQ], BF16, tag=f"kT{c}", name=f"kT{c}")
                  for c in range(2)]
            vv = [pp.tile([128, NH_LOC, HD + 1], PROBS_DT, tag=f"v{s}",
                          name=f"v{s}") for s in range(SEQ // SC)]

            # warm-up collective: completes ncfw comm-init before the first
            # real AllGather's doorbell rings (mid-init triggers stall ~340us)
            nc.gpsimd.collective_compute(
                "AllGather", mybir.AluOpType.bypass,
                replica_groups=[[0, 1, 2, 3], [4, 5, 6, 7]],
                ins=[warm_in[:]], outs=[warm_out[:]])

            # ---- input DMAs (x via ACT queue, rest via SP queue) -----------
            for i in range(NIC):
                nc.scalar.dma_start(out=x_t[i][:],
                                    in_=xT[i * 128:(i + 1) * 128, :])
            for i in range(NIC):
                nc.sync.dma_start(out=w3_t[i][:],
                                  in_=w3T[i * 128:(i + 1) * 128, :])
            nc.sync.dma_start(out=bqk_t[:], in_=bqk[:])
            nc.sync.dma_start(out=bv_bt[:],
                              in_=bvB[0:1, :].to_broadcast((128, DCOL)))
            nc.sync.dma_start(out=mask_t[:], in_=masks[:, :])
            for i in range(NIC):
                nc.sync.dma_start(out=wm_t[i][:],
                                  in_=wmT[i * 128:(i + 1) * 128, :])
            nc.sync.dma_start(out=bm_t[:], in_=bmc[:])

            # ---- phases 1-3 interleaved per seq block ----------------------
            # attention for block j needs only q[j], k/v[<=j]; emitting
            # projections and attention per block keeps ScalarE exp running
            # alongside phase-1 matmuls instead of after them.
            def proj_qk(sj):
                for qk in range(2):          # 0 = q, 1 = k
                    dst = (qT, kT)[qk]
                    for oc in range(2):
                        pq = pmm.tile([128, SJ], F32, tag="pmm", name="pq")
                        for ic in range(NIC):
                            nc.tensor.matmul(
                                out=pq[:],
                                lhsT=w3_t[ic][:, qk * DCOL + oc * 128:
                                              qk * DCOL + (oc + 1) * 128],
                                rhs=x_t[ic][:, sj_slice(sj)],
                                start=(ic == 0), stop=(ic == NIC - 1))
                        nc.vector.tensor_scalar_add(
                            out=dst[oc][:, sj_slice(sj)],
                            in0=pq[:],
                            scalar1=bqk_t[:, qk * 2 + oc:qk * 2 + oc + 1])

            def proj_v(sj):
                for s in range(sj * NDIAG, (sj + 1) * NDIAG):
                    nc.vector.memset(vv[s][:, :, HD:HD + 1], 1.0)
                    pv = pmm.tile([128, DCOL], F32, tag="pmm", name="pv")
                    for ic in range(NIC):
                        nc.tensor.matmul(
                            out=pv[:],
                            lhsT=x_t[ic][:, s * SC:(s + 1) * SC],
                            rhs=w3_t[ic][:, 2 * DCOL:3 * DCOL],
                            start=(ic == 0), stop=(ic == NIC - 1))
                    nc.vector.tensor_add(
                        out=vv[s][:, :, 0:HD],
                        in0=pv[:].rearrange("p (h d) -> p h d", h=NH_LOC),
                        in1=bv_bt[:].rearrange("p (h d) -> p h d", h=NH_LOC))

            def attention(j):
                noff = j * NDIAG
                nt = noff + NDIAG
                for ch in range(2):      # head pair (2ch, 2ch+1)
                    prt = []
                    for t in range(nt):
                        co = 0 if t < noff else (t - noff) * SC
                        w = SJ - co
                        # both heads' scoresT chunk via 64x128 PE row tiling
                        sp2 = ps.tile([128, 2 * SJ], F32, tag="ps", name="sp2")
                        nc.tensor.matmul(
                            out=sp2[:, 0:w],
                            lhsT=kT[ch][0:HD, t * SC:(t + 1) * SC],
                            rhs=qT[ch][0:HD, j * SJ + co:(j + 1) * SJ],
                            start=True, stop=True, tile_position=(0, 0))
                        nc.tensor.matmul(
                            out=sp2[:, SJ:SJ + w],
                            lhsT=kT[ch][HD:128, t * SC:(t + 1) * SC],
                            rhs=qT[ch][HD:128, j * SJ + co:(j + 1) * SJ],
                            start=True, stop=True, tile_position=(64, 0))
                        pr = probs_pool.tile([128, 2, SJ], PROBS_DT, tag="pr",
                                             name="pr")
                        nc.scalar.activation(
                            out=pr[:, :, 0:w],
                            in_=sp2[:].rearrange("p (b s) -> p b s", b=2)[:, :, 0:w],
                            func=mybir.ActivationFunctionType.Exp,
                            scale=SCALE)
                        if t >= noff:  # triangular mask on diagonal SC cols
                            nc.vector.tensor_mul(out=pr[:, 0, 0:SC],
                                                 in0=pr[:, 0, 0:SC], in1=mask_t[:])
                            nc.vector.tensor_mul(out=pr[:, 1, 0:SC],
                                                 in0=pr[:, 1, 0:SC], in1=mask_t[:])
                        prt.append((pr, co, w))
                    for hh in range(2):
                        h = 2 * ch + hh
                        att_ps = pa.tile([128, SJ], F32, tag="pa", name="att_ps")
                        for t, (pr, co, w) in enumerate(prt):
                            nc.tensor.matmul(
                                out=att_ps[0:HD + 1, co:co + w],
                                lhsT=vv[t][:, h, :],
                                rhs=pr[:, hh, 0:w],
                                start=(t == 0), stop=(t == nt - 1))
                        # normalize rows by the ones-column sum
                        den = wp.tile([128, SJ], F32, tag="den", name="den")
                        nc.vector.tensor_copy(den[HD:HD + 1, :],
                                              att_ps[HD:HD + 1, :])
                        ridx = h * NSJ + j
                        nc.sync.dma_start(out=rec_dram[ridx:ridx + 1, :],
                                          in_=den[HD:HD + 1, :])
                        rec = wp.tile([128, SJ], F32, tag="rec", name="rec")
                        nc.sync.dma_start(
                            out=rec[0:HD, :],
                            in_=rec_dram[ridx:ridx + 1, :].to_broadcast((HD, SJ)))
                        nc.vector.reciprocal_approx_fast(out=rec[0:HD, :],
                                                         in_=rec[0:HD, :])
                        atn = wp.tile([128, SJ], BF16, tag="atn", name="atn")
                        nc.vector.tensor_mul(out=atn[0:HD, :],
                                             in0=att_ps[0:HD, :],
                                             in1=rec[0:HD, :])
                        nc.sync.dma_start(
                            out=att_loc[j][h * HD:(h + 1) * HD, :],
                            in_=atn[0:HD, :])
                # gather all 16 heads' attention for this seq block
                nc.gpsimd.collective_compute(
                    "AllGather", mybir.AluOpType.bypass,
                    replica_groups=[[0, 1, 2, 3], [4, 5, 6, 7]],
                    ins=[att_loc[j][:]], outs=[att_g[j][:]])
                agt = ag_pool.tile([128, NIC, SJ], BF16, tag="ag",
                                   name=f"ag{j}", bufs=2)
                nc.sync.dma_start(
                    out=agt[:],
                    in_=att_g[j][:].rearrange("(i p) s -> p i s", p=128))
                for oc in range(2):
                    pout = pmm.tile([128, SJ], F32, tag="pmm", name="pout")
                    for ic in range(NIC):
                        nc.tensor.matmul(
                            out=pout[:],
                            lhsT=wm_t[ic][:, oc * 128:(oc + 1) * 128],
                            rhs=agt[:, ic, :],
                            start=(ic == 0), stop=(ic == NIC - 1))
                    osb = wp.tile([128, SJ], F32, tag="osb", name="osb")
                    nc.vector.tensor_scalar_add(out=osb[:], in0=pout[:],
                                                scalar1=bm_t[:, oc:oc + 1])
                    nc.sync.dma_start(
                        out=out[oc * 128:(oc + 1) * 128, sj_slice(j)],
                        in_=osb[:])

            for sj in range(NSJ):
                proj_qk(sj)
                proj_v(sj)
            for j in range(NSJ):
                attention(j)

    nc.compile()
    return nc


_NC_CACHE = None


def _get_program():
    global _NC_CACHE
    if _NC_CACHE is None:
        _NC_CACHE = build_program()
    return _NC_CACHE


def make_in_maps(x, Wq, bq, Wk, bk, Wv, bv, Wm, bm):
    import ml_dtypes
    bf16 = ml_dtypes.bfloat16
    xT = [np.ascontiguousarray(x[b].T).astype(bf16) for b in range(2)]
    p = np.arange(128)[:, None]
    f = np.arange(SC)[None, :]
    masks = (f >= p).astype(bf16)
    in_maps = []
    for core in range(8):
        b, g = core // 4, core % 4
        gr = slice(g * DCOL, (g + 1) * DCOL)
        w3 = np.concatenate([Wq[gr, :].T, Wk[gr, :].T, Wv[gr, :].T],
                            axis=1).astype(bf16)
        bqk_c = np.stack([bq[gr][0:128], bq[gr][128:256],
                          bk[gr][0:128], bk[gr][128:256]],
                         axis=1).astype(np.float32)
        bmc = np.stack([bm[gr][0:128], bm[gr][128:256]],
                       axis=1).astype(np.float32)
        in_maps.append({
            "xT": np.ascontiguousarray(xT[b]),
            "w3T": np.ascontiguousarray(w3),
            "wmT": np.ascontiguousarray(Wm[gr, :].T).astype(bf16),
            "bqk": np.ascontiguousarray(bqk_c),
            "bvB": np.ascontiguousarray(bv[gr].reshape(1, DCOL)).astype(np.float32),
            "bmc": np.ascontiguousarray(bmc),
            "masks": masks,
        })
    return in_maps


def assemble_output(results):
    out = np.empty((2, SEQ, HID), np.float32)
    for b in range(2):
        outT = np.concatenate(
            [results[b * 4 + g]["out"] for g in range(4)], axis=0)
        out[b] = outT.T
    return out


def kernel(x, Wq, bq, Wk, bk, Wv, bv, Wm, bm, _trace=False):
    nc = _get_program()
    in_maps = make_in_maps(np.asarray(x, np.float32), np.asarray(Wq),
                           np.asarray(bq), np.asarray(Wk), np.asarray(bk),
                           np.asarray(Wv), np.asarray(bv), np.asarray(Wm),
                           np.asarray(bm))
    res = run_bass_kernel_spmd(nc, in_maps, list(range(8)), trace=_trace)
    out = assemble_output(res.results)
    if _trace:
        return out, res
    return out
